# revision 28
# baseline (speedup 1.0000x reference)
"""Trainium2 Bass kernel v3 for nn_Attention_72224170050112.

Multi-head attention (B=4, T=2048, D=1024, H=16, HD=64) on 8 NeuronCores.
Sharding: 4-way data-parallel over batch x 2-way tensor-parallel over heads
(core c: batch c//2, head-group c%2). Host sums the two w_proj partials per
batch and adds b_proj.

v4 changes vs v2.1 (trace: PE busy only 65%, HAM re-throttled to 1.2GHz at
every block boundary, 85us serial gpsimd norm chain in the tail):
- Normalization rework: the per-block softmax epilogue is now
  den[33,512] <- DVE copies of the two PV 'ones'-row denominators (rows
  0 and 32; engine writes need 32-aligned partition starts), ONE DVE
  reciprocal, two gpsimd partition_broadcasts (gpsimd now runs only this
  op, so its ucode lib loads once -- no LOAD/UNLOAD thrash; its cpu0
  reads the source, so hl1's row first hops to partition 0 via ScalarE),
  and two DVE multiplies that read the PV PSUM directly. Kills: the
  gpsimd norm multiplies, one of the two 3.3us DVE reciprocals, and the
  PSUM->SBUF numerator drains. (A v3 attempt broadcast via a tiny f32 PE
  matmul instead: the fp32 LOW_HIGH pairs cost 34us of PE and, worse,
  the matmul waited ~4us/block on the reciprocal INSIDE the PE queue --
  the PE reorders only LDWEIGHTS, so the whole stream stalled.)
- Re-paced weaving: PV matmuls pop 2 per j-chunk across the whole next
  s_loop (PE never runs dry at block boundaries -> HAM stays at 2.4GHz);
  the epilogue pops at j=0..4 two blocks later, so the 3.3us reciprocal
  sits mid-block in DVE's in-order queue instead of blocking the next
  pair's qt/kt bias-adds at the boundary; fillers pop at j%4==1.
- Projection chains spread across the last pair: proj(tb) weaves into
  s_loop(3,tb+1) right after ctx(3,tb) is normalized, instead of 24+8
  chains serialized after the final block. wproj DMA moves to pair 2.
- exp split 12/16 ScalarE (true Exp) + 4/16 DVE (Schraudolph bf16
  bit-trick) at j in (3,7,11,14); final block splits 8/8.
- x arrives over four otherwise-idle DMA queues (scalar/gpsimd/vector/
  tensor) so all four 512-token blocks land by ~4.5us.
"""

import numpy as np
import ml_dtypes

B, T, D, H = 4, 2048, 1024, 16
HD = D // H  # 64
NCORES = 8
G = D // 2  # feature dims per head-group = 512
NH = H // 2  # heads per core = 8
CC = D // 128  # 8 contraction chunks for QKV
GC = G // 128  # 4 head pairs per core
TB = T // 512  # 4 t/q blocks
TCH = T // 128  # 16 t-chunks (attention k chunks)

LOG2E = 1.4426950408889634
SCHR_SLOPE = 128.0 * LOG2E / 8.0
SCHR_C = 128.0 * 127.0 - 7.0 + 0.5  # min-max wiggle correction, trunc->round
DVE_JS = (2, 5, 8, 11, 14)  # j-chunks whose exp runs on VectorE (5/16)

_cache = {}


def _build():
    from collections import deque

    import concourse.bacc as bacc
    import concourse.tile as tile
    from concourse import mybir

    dt = mybir.dt
    f32, bf16, i16 = dt.float32, dt.bfloat16, dt.int16
    AF = mybir.ActivationFunctionType
    Alu = mybir.AluOpType

    nc = bacc.Bacc(
        "TRN2",
        target_bir_lowering=False,
        debug=False,
        enable_asserts=True,
        num_devices=NCORES,
    )
    xT = nc.dram_tensor("xT", [D, T], bf16, kind="ExternalInput").ap()
    wq = nc.dram_tensor("wq", [D, G], bf16, kind="ExternalInput").ap()
    wk = nc.dram_tensor("wk", [D, G], bf16, kind="ExternalInput").ap()
    wv = nc.dram_tensor("wv", [D, G], bf16, kind="ExternalInput").ap()
    bqkv = nc.dram_tensor("bqkv", [3, G], f32, kind="ExternalInput").ap()
    wp = nc.dram_tensor("wp", [G, D], bf16, kind="ExternalInput").ap()
    ones = nc.dram_tensor("ones", [128, NH], f32, kind="ExternalInput").ap()
    outT = nc.dram_tensor("outT", [D, T], bf16, kind="ExternalOutput").ap()

    W_APS = {0: wq, 1: wk, 2: wv}
    # xT viewed as [128, CC, T] for one-DMA resident load
    xT_v = xT.rearrange("(c p) t -> p c t", p=128)
    wp_v = wp.rearrange("(c p) d -> p c d", p=128)

    with tile.TileContext(nc) as tc:
        with (
            tc.tile_pool(name="store", bufs=GC) as store,
            tc.tile_pool(name="vaugp", bufs=TB) as vaugp,
            tc.tile_pool(name="xres", bufs=1) as xres,
            tc.tile_pool(name="misc", bufs=4) as misc,
            tc.tile_pool(name="nrm", bufs=3) as nrm,
            tc.tile_pool(name="stage", bufs=4) as stage,
            tc.tile_pool(name="pm", bufs=2, space="PSUM") as pm,
            tc.tile_pool(name="pq", bufs=2, space="PSUM") as pq,
            tc.tile_pool(name="pvp", bufs=2, space="PSUM") as pvp,
        ):
            ctx_t = [
                store.tile([128, T], bf16, tag="ctx", name=f"ctx{i}")
                for i in range(GC)
            ]
            vaug_tb = [
                vaugp.tile(
                    [128, 4, NH, 65], bf16, tag="vaug", name=f"vaug{i}"
                )
                for i in range(TB)
            ]

            def vaug_sl(j):
                return vaug_tb[j // 4][:, j % 4, :, :]
            ones_bf = misc.tile([128, NH], bf16, tag="ones16")
            nc.gpsimd.dma_start(ones_bf[:], ones)  # f32 -> bf16 cast DMA
            # persistent denominator-gather tiles: engine writes must start
            # at 32-aligned partitions, so hl0 lands on partition 0 and hl1
            # on partition 32; rows 1-31 are memset once to 1.0 so the
            # one-shot [33,512] reciprocal stays finite there (unread)
            den_ab = [
                misc.tile([33, 512], f32, tag=f"den{i}", name=f"den{i}")
                for i in range(3)
            ]
            for t in den_ab:
                nc.vector.memset(t[:], 1.0)
            # warm the ScalarE Exp table during the startup DMA wait
            warm = misc.tile([1, 2], f32, tag="warm")
            nc.vector.memset(warm[:], 0.0)
            nc.scalar.activation(warm[:], warm[:], AF.Exp, scale=0.125)
            # PE warm-up spin: ~70 tiny matmuls on a memset tile keep the
            # PE's HAM activity window busy while the x/weight DMAs land,
            # so real work starts at 2.4GHz instead of 1.2GHz (the clock
            # gate needs ~3.4us of sustained activity to open, and the
            # kernel otherwise idles the PE for its first ~7us)
            wsrc = misc.tile([1, 64], bf16, tag="wsrc")
            nc.vector.memset(wsrc[:], 0.0)
            wps = pq.tile([1, 64], f32, tag="qkv", name="wps")
            for _ in range(70):
                nc.tensor.matmul(wps[:], wsrc[:, 0:1], wsrc[:], start=True, stop=True)

            # resident x^T, one tile per 512-token block so chains gate
            # on per-block DMA completion; block 0 is split into two halves
            # so the very first v-chain starts after a 4KB/partition DMA
            x0h = [
                xres.tile([128, CC, 256], bf16, tag=f"x0h{i}", name=f"x0h{i}")
                for i in range(2)
            ]
            x_tb = [None] + [
                xres.tile([128, CC, 512], bf16, tag=f"x{h}", name=f"x{h}")
                for h in range(1, TB)
            ]

            def x_sl(tb, i):
                """x slice for key-chunk i (128 tokens) of block tb."""
                if tb == 0:
                    return x0h[i // 2][:, :, (i % 2) * 128 : (i % 2) * 128 + 128]
                return x_tb[tb][:, :, i * 128 : (i + 1) * 128]

            for j in range(TCH):
                nc.vector.tensor_copy(vaug_sl(j)[:, :, 64], ones_bf[:])

            kt_cur, qt_cur = [None], [None]
            kt_nxt, qt_nxt = [None], [None]

            attn_pools = (
                tc.tile_pool(name="pw", bufs=7),
                tc.tile_pool(name="pwproj", bufs=1),
                tc.tile_pool(name="pkq", bufs=2),
                tc.tile_pool(name="pp", bufs=20),
                tc.tile_pool(name="pbias", bufs=16),
            )
            pwp, pwpr, pkq, ppool, pbias = (
                pl.__enter__() for pl in attn_pools
            )

            def load_w(o, p):
                """One DMA for all CC chunks of weight o, pair p: tile
                [128, CC, 128]; slice [:, cc, :] is the lhsT for chunk cc."""
                w_ap = W_APS[o].rearrange("(c p) g -> p c g", p=128)
                t = pwp.tile([128, CC, 128], bf16, tag="w", name=f"w{o}_{p}")
                nc.sync.dma_start(
                    t[:], w_ap[:, :, p * 128 : (p + 1) * 128]
                )
                bt = pbias.tile([128, 1], f32, tag="bias", name=f"b{o}_{p}")
                nc.sync.dma_start(bt[:], bqkv[o, p * 128 : (p + 1) * 128])
                return t, bt

            def make_fillers(p):
                """Projection work for pair p: per tb, a q-chain, k-chain
                (token-moving) and a v-group (4 key-chunks, x-stationary,
                natural layout)."""
                wv_t, _ = load_w(2, p)
                wk_t, bt_k = load_w(1, p)
                wq_t, bt_q = load_w(0, p)
                kt = pkq.tile([128, T], bf16, tag="kt", name=f"kt{p}")
                qt = pkq.tile([128, T], bf16, tag="qt", name=f"qt{p}")
                kt_nxt[0], qt_nxt[0] = kt, qt
                h0 = 2 * p

                def qk_chain(o, tb):
                    def f():
                        wt = (wk_t, wq_t)[o]
                        ps = pq.tile(
                            [128, 512], f32, tag="qkv", name=f"ps{p}_{o}_{tb}"
                        )
                        if tb == 0:
                            for half in range(2):
                                for cc in range(CC):
                                    nc.tensor.matmul(
                                        ps[:, half * 256 : (half + 1) * 256],
                                        wt[:, cc, :],
                                        x0h[half][:, cc, :],
                                        start=(cc == 0),
                                        stop=(cc == CC - 1),
                                    )
                        else:
                            for cc in range(CC):
                                nc.tensor.matmul(
                                    ps[:],
                                    wt[:, cc, :],
                                    x_tb[tb][:, cc, :],
                                    start=(cc == 0),
                                    stop=(cc == CC - 1),
                                )
                        dst = (kt, qt)[o]
                        bt = (bt_k, bt_q)[o]
                        nc.vector.tensor_scalar_add(
                            dst[:, tb * 512 : (tb + 1) * 512], ps[:], bt[:]
                        )

                    return f

                def v_group(tb):
                    def f():
                        vps = pq.tile(
                            [128, 4, 128], f32, tag="qkv", name=f"vps{p}_{tb}"
                        )
                        for i in range(4):
                            xsl = x_sl(tb, i)
                            for cc in range(CC):
                                nc.tensor.matmul(
                                    vps[:, i, :],
                                    xsl[:, cc, :],
                                    wv_t[:, cc, :],
                                    start=(cc == 0),
                                    stop=(cc == CC - 1),
                                )
                        dst = vaug_tb[tb][:, :, h0 : h0 + 2, 0:64]
                        src = vps[:].rearrange("p k (h c) -> p k h c", c=64)
                        nc.vector.tensor_copy(dst, src)

                    return f

                out = []
                for tb in range(TB):
                    out.append(v_group(tb))
                    out.append(qk_chain(0, tb))
                    out.append(qk_chain(1, tb))
                return out

            fillers = deque()
            pv_backlog = deque()  # PV matmul closures (32 per block)
            epi_backlog = deque()  # released epilogue (+pair-3 proj) closures
            epi_queue = deque()  # per-block closure lists awaiting release

            def s_loop(
                p, qb, mm_rate=2, epi_lo=0, epi_rate=1, epi_stride=1,
                inline_pv=None,
            ):
                """S + exp for (pair p, q-block qb), weaving in backlogged
                PV matmuls (mm_rate per j), the delayed normalization
                epilogue (+pair-3 proj chains; epi_rate per j from j=epi_lo),
                and one projection filler at j%4==1. exp runs on ScalarE
                except DVE_JS chunks (VectorE Schraudolph fast-exp).
                inline_pv: callback (j, P_tiles) for the final q-block."""
                kt, qt = kt_cur[0], qt_cur[0]
                qs = slice(qb * 512, (qb + 1) * 512)
                # final block: split exp evenly so ACT+DVE drain together
                dve_js = DVE_JS if inline_pv is None else tuple(range(1, TCH, 2))
                P_tiles = []
                for j in range(TCH):
                    sp = pm.tile(
                        [128, 1024], f32, tag="mm", name=f"sp{p}_{qb}_{j}"
                    )
                    for hl in range(2):
                        rows = slice(hl * 64, hl * 64 + 64)
                        nc.tensor.matmul(
                            sp[:, hl * 512 : (hl + 1) * 512],
                            kt[rows, j * 128 : (j + 1) * 128],
                            qt[rows, qs],
                            start=True,
                            stop=True,
                            tile_position=(hl * 64, 0),
                        )
                    P = ppool.tile(
                        [128, 1024], bf16, tag="p", name=f"P{p}_{qb}_{j}"
                    )
                    if j in dve_js:
                        nc.vector.tensor_scalar(
                            P[:].bitcast(i16),
                            sp[:],
                            SCHR_SLOPE,
                            SCHR_C,
                            Alu.mult,
                            Alu.add,
                        )
                    else:
                        nc.scalar.activation(P[:], sp[:], AF.Exp, scale=0.125)
                    P_tiles.append(P)
                    if inline_pv is not None:
                        inline_pv(j, P_tiles)
                    for _ in range(mm_rate):
                        if pv_backlog:
                            pv_backlog.popleft()()
                    if j >= epi_lo and (j - epi_lo) % epi_stride == 0:
                        for _ in range(epi_rate):
                            if epi_backlog:
                                epi_backlog.popleft()()
                    if j % 4 == 1 and fillers:
                        fillers.popleft()()
                return P_tiles

            def make_pv(p, qb, P_tiles):
                """P@V accumulation chains for (p, qb) plus the deferred
                normalization epilogue. Returns (mm_items, epi_items);
                epi items: ScalarE den copies -> DVE reciprocal [2,512] ->
                f32 PE broadcast matmul -> ScalarE PSUM->SBUF copy -> DVE
                norm multiplies reading the PV PSUM directly."""
                qs = slice(qb * 512, (qb + 1) * 512)
                pvt = [
                    pvp.tile([65, 512], f32, tag="pv", name=f"pv{p}_{qb}_{i}")
                    for i in range(2)
                ]

                def mk_mm(hl, j):
                    def f():
                        hg = 2 * p + hl
                        nc.tensor.matmul(
                            pvt[hl][0:65, :],
                            vaug_sl(j)[:, hg, :],
                            P_tiles[j][:, hl * 512 : (hl + 1) * 512],
                            start=(j == 0),
                            stop=(j == TCH - 1),
                        )

                    return f

                box = {}
                den = den_ab[(p * TB + qb) % 3]

                def e_drain():
                    # ScalarE PSUM->SBUF drains free the PV accumulator
                    # banks immediately -- the next-next block's PV start
                    # no longer WAR-waits on the reciprocal->broadcast->
                    # norm chain
                    pvs = [
                        nrm.tile([65, 512], f32, tag=f"pvs{i}", name=f"pvs{i}")
                        for i in range(2)
                    ]
                    for hl in range(2):
                        nc.scalar.copy(pvs[hl][:], pvt[hl][:])
                    box["pvs"] = pvs

                def e_den():
                    pvs = box["pvs"]
                    nc.vector.tensor_copy(den[0:1, :], pvs[0][64:65, :])
                    nc.vector.tensor_copy(den[32:33, :], pvs[1][64:65, :])

                def e_recip():
                    rcp = nrm.tile([33, 512], f32, tag="rcp")
                    nc.vector.reciprocal(rcp[:], den[:])
                    # gpsimd's broadcast ucode reads the source on Q7 core
                    # 0 (partitions 0-15), so hl1's reciprocal row hops to
                    # partition 0 via ScalarE
                    rcp_b = nrm.tile([1, 512], f32, tag="rcpb")
                    nc.scalar.copy(rcp_b[:], rcp[32:33, :])
                    box["rcp"], box["rcp_b"] = rcp, rcp_b

                def e_bcast():
                    # gpsimd runs ONLY this op -> its ucode library loads
                    # once, no LOAD/UNLOAD swapping
                    rsb0 = nrm.tile([64, 512], f32, tag="rsb0")
                    rsb1 = nrm.tile([64, 512], f32, tag="rsb1")
                    nc.gpsimd.partition_broadcast(rsb0[:], box["rcp"][0:1, :])
                    nc.gpsimd.partition_broadcast(rsb1[:], box["rcp_b"][:])
                    box["rsb"] = (rsb0, rsb1)

                def e_norm():
                    # on gpsimd: all-SBUF operands, and it offloads ~2us/
                    # block from the queue-saturated DVE (costs a ucode
                    # lib swap around the broadcasts, but gpsimd is idle)
                    for hl in range(2):
                        nc.gpsimd.tensor_tensor(
                            ctx_t[p][hl * 64 : hl * 64 + 64, qs],
                            box["pvs"][hl][0:64, :],
                            box["rsb"][hl][:],
                            Alu.mult,
                        )

                mm_items = []
                for j in range(TCH):
                    for hl in range(2):
                        mm_items.append(mk_mm(hl, j))
                epi_items = [e_drain, e_den, e_recip, e_bcast, e_norm]
                return mm_items, epi_items

            wpt_box = [None]

            def load_wproj():
                wpt_box[0] = pwpr.tile(
                    [128, GC, D], bf16, tag="wproj", name="wpt"
                )
                nc.sync.dma_start(wpt_box[0][:], wp_v)

            def proj_chain(oc, tb):
                def f():
                    wpt = wpt_box[0]
                    ps = pq.tile(
                        [128, 512], f32, tag="qkv", name=f"cps{oc}_{tb}"
                    )
                    for cc in range(GC):
                        nc.tensor.matmul(
                            ps[:],
                            wpt[:, cc, oc * 128 : (oc + 1) * 128],
                            ctx_t[cc][:, tb * 512 : (tb + 1) * 512],
                            start=(cc == 0),
                            stop=(cc == GC - 1),
                        )
                    ost = stage.tile([128, 512], bf16, tag="ost")
                    if tb == TB - 1:
                        nc.scalar.copy(ost[:], ps[:])  # tail: ScalarE is idle
                    else:
                        nc.vector.tensor_copy(ost[:], ps[:])
                    nc.sync.dma_start(
                        outT[
                            oc * 128 : (oc + 1) * 128, tb * 512 : (tb + 1) * 512
                        ],
                        ost[:],
                    )

                return f

            # ---------------- pipeline ----------------
            fillers0 = make_fillers(0)  # issues pair-0 weight DMAs first
            # weights/biases ride the sync queue; x spreads over the
            # scalar/gpsimd queues plus sync (behind the small weight
            # DMAs) so all four blocks land by ~5us
            nc.scalar.dma_start(x0h[0][:], xT_v[:, :, 0:256])
            nc.gpsimd.dma_start(x0h[1][:], xT_v[:, :, 256:512])
            x_queues = (None, nc.scalar, nc.gpsimd, nc.sync)
            for h in range(1, TB):
                sl = slice(h * 512, (h + 1) * 512)
                x_queues[h].dma_start(x_tb[h][:], xT_v[:, :, sl])
            for f in fillers0:
                f()
            kt_cur[0], qt_cur[0] = kt_nxt[0], qt_nxt[0]
            for p in range(GC):
                if p + 1 < GC:
                    fillers.extend(make_fillers(p + 1))
                if p == 2:
                    load_wproj()
                for qb in range(TB):
                    last = p == GC - 1 and qb == TB - 1
                    # release delayed closure lists: pairs 0-2 keep them
                    # two blocks back (popped 1/j from j=0; their PV mms
                    # finished issuing a block earlier). Pair 3 drains PV
                    # at 4/j (done by j=7) and pops epi+proj 2/j from j=8,
                    # one block back, so proj(3,tb) follows norm(3,tb)
                    # inside the same deque.
                    if p < GC - 1:
                        if len(epi_queue) >= 2:
                            epi_backlog.extend(epi_queue.popleft())
                        rates = dict(mm_rate=2, epi_lo=0, epi_rate=1, epi_stride=3)
                    else:
                        while epi_queue:
                            epi_backlog.extend(epi_queue.popleft())
                        rates = dict(mm_rate=4, epi_lo=8, epi_rate=2, epi_stride=1)
                    if not last:
                        P_tiles = s_loop(p, qb, **rates)
                        mm_items, epi_items = make_pv(p, qb, P_tiles)
                        pv_backlog.extend(mm_items)
                        if p == GC - 1:
                            # ctx(3,qb) projections ride the same deque,
                            # strictly after norm(3,qb)
                            epi_items = epi_items + [
                                proj_chain(oc, qb) for oc in range(CC)
                            ]
                        epi_queue.append(epi_items)
                        continue
                    # final q-block: PV(3,2) drains at 4/j (done j=7),
                    # epi(3,2)+proj(tb2) pop 2/j from j=8; own PV inlines
                    # from j=5 (after norm(3,1) issued in the previous
                    # block, freeing its pvp slot); epilogue at j=15,
                    # proj(tb3) in the tail
                    pv_state = {}

                    def inline(j, P_tiles):
                        if j == 5:
                            mm_items, epi_items = make_pv(p, qb, P_tiles)
                            for jj in range(TCH):
                                pv_state[jj] = mm_items[2 * jj : 2 * jj + 2]
                            pv_state["epi"] = epi_items
                            for jj in range(5):
                                for f in pv_state[jj]:
                                    f()
                        if j >= 5:
                            for f in pv_state[j]:
                                f()
                        if j == TCH - 1:
                            for f in pv_state["epi"]:
                                f()

                    s_loop(p, qb, inline_pv=inline, **rates)
                if p + 1 < GC:
                    kt_cur[0], qt_cur[0] = kt_nxt[0], qt_nxt[0]
            # ---------------- tail: remaining proj chains ----------------
            while epi_backlog:
                epi_backlog.popleft()()
            for oc in range(CC):
                proj_chain(oc, TB - 1)()
            for pl in reversed(attn_pools):
                pl.__exit__(None, None, None)

    nc.compile()
    return nc


def _get_nc():
    if "nc" not in _cache:
        _cache["nc"] = _build()
    return _cache["nc"]


def make_in_maps(x, w_qkv, b_qkv, w_proj):
    """Host-side sharding: per-core input dict."""
    BF = ml_dtypes.bfloat16
    x = np.asarray(x, dtype=np.float32)
    w_qkv = np.asarray(w_qkv, dtype=np.float32)
    b_qkv = np.asarray(b_qkv, dtype=np.float32)
    ones = np.ones((128, NH), dtype=np.float32)
    in_maps = []
    for c in range(NCORES):
        b, g = divmod(c, 2)
        sl = slice(g * G, (g + 1) * G)
        in_maps.append(
            {
                "xT": np.ascontiguousarray(x[b].T).astype(BF),
                "wq": np.ascontiguousarray(w_qkv[:, 0 * D : 1 * D][:, sl]).astype(BF),
                "wk": np.ascontiguousarray(w_qkv[:, 1 * D : 2 * D][:, sl]).astype(BF),
                "wv": np.ascontiguousarray(w_qkv[:, 2 * D : 3 * D][:, sl]).astype(BF),
                "bqkv": np.stack(
                    [
                        b_qkv[0 * D : 1 * D][sl],
                        b_qkv[1 * D : 2 * D][sl],
                        b_qkv[2 * D : 3 * D][sl],
                    ]
                ).astype(np.float32),
                "wp": np.ascontiguousarray(
                    np.asarray(w_proj, np.float32)[sl, :]
                ).astype(BF),
                "ones": ones,
            }
        )
    return in_maps


def unshard(results, b_proj):
    b_proj = np.asarray(b_proj, dtype=np.float32)
    out = np.empty((B, T, D), dtype=np.float32)
    for b in range(B):
        s = (
            results[2 * b]["outT"].astype(np.float32)
            + results[2 * b + 1]["outT"].astype(np.float32)
        )  # [D, T]
        out[b] = s.T + b_proj
    return out


def kernel(x, w_qkv, b_qkv, w_proj, b_proj):
    from concourse.bass_utils import run_bass_kernel_spmd

    nc = _get_nc()
    in_maps = make_in_maps(x, w_qkv, b_qkv, w_proj)
    res = run_bass_kernel_spmd(nc, in_maps, core_ids=list(range(NCORES)))
    return unshard(res.results, b_proj)


# revision 29
# speedup vs baseline: 1.4447x; 1.4447x over previous
"""Trainium2 Bass kernel v3 for nn_Attention_72224170050112.

Multi-head attention (B=4, T=2048, D=1024, H=16, HD=64) on 8 NeuronCores.
Sharding: 4-way data-parallel over batch x 2-way tensor-parallel over heads
(core c: batch c//2, head-group c%2). Host sums the two w_proj partials per
batch and adds b_proj.

v4 changes vs v2.1 (trace: PE busy only 65%, HAM re-throttled to 1.2GHz at
every block boundary, 85us serial gpsimd norm chain in the tail):
- Normalization rework: the per-block softmax epilogue is now
  den[33,512] <- DVE copies of the two PV 'ones'-row denominators (rows
  0 and 32; engine writes need 32-aligned partition starts), ONE DVE
  reciprocal, two gpsimd partition_broadcasts (gpsimd now runs only this
  op, so its ucode lib loads once -- no LOAD/UNLOAD thrash; its cpu0
  reads the source, so hl1's row first hops to partition 0 via ScalarE),
  and two DVE multiplies that read the PV PSUM directly. Kills: the
  gpsimd norm multiplies, one of the two 3.3us DVE reciprocals, and the
  PSUM->SBUF numerator drains. (A v3 attempt broadcast via a tiny f32 PE
  matmul instead: the fp32 LOW_HIGH pairs cost 34us of PE and, worse,
  the matmul waited ~4us/block on the reciprocal INSIDE the PE queue --
  the PE reorders only LDWEIGHTS, so the whole stream stalled.)
- Re-paced weaving: PV matmuls pop 2 per j-chunk across the whole next
  s_loop (PE never runs dry at block boundaries -> HAM stays at 2.4GHz);
  the epilogue pops at j=0..4 two blocks later, so the 3.3us reciprocal
  sits mid-block in DVE's in-order queue instead of blocking the next
  pair's qt/kt bias-adds at the boundary; fillers pop at j%4==1.
- Projection chains spread across the last pair: proj(tb) weaves into
  s_loop(3,tb+1) right after ctx(3,tb) is normalized, instead of 24+8
  chains serialized after the final block. wproj DMA moves to pair 2.
- exp split 12/16 ScalarE (true Exp) + 4/16 DVE (Schraudolph bf16
  bit-trick) at j in (3,7,11,14); final block splits 8/8.
- x arrives over four otherwise-idle DMA queues (scalar/gpsimd/vector/
  tensor) so all four 512-token blocks land by ~4.5us.
"""

import numpy as np
import ml_dtypes

B, T, D, H = 4, 2048, 1024, 16
HD = D // H  # 64
NCORES = 8
G = D // 2  # feature dims per head-group = 512
NH = H // 2  # heads per core = 8
CC = D // 128  # 8 contraction chunks for QKV
GC = G // 128  # 4 head pairs per core
TB = T // 512  # 4 t/q blocks
TCH = T // 128  # 16 t-chunks (attention k chunks)

LOG2E = 1.4426950408889634
SCHR_SLOPE = 128.0 * LOG2E / 8.0
SCHR_C = 128.0 * 127.0 - 7.0 + 0.5  # min-max wiggle correction, trunc->round
DVE_JS = (2, 5, 8, 11, 14)  # j-chunks whose exp runs on VectorE (5/16)

_cache = {}


def _build():
    from collections import deque

    import concourse.bacc as bacc
    import concourse.tile as tile
    from concourse import mybir

    dt = mybir.dt
    f32, bf16, i16 = dt.float32, dt.bfloat16, dt.int16
    AF = mybir.ActivationFunctionType
    Alu = mybir.AluOpType

    nc = bacc.Bacc(
        "TRN2",
        target_bir_lowering=False,
        debug=False,
        enable_asserts=True,
        num_devices=NCORES,
    )
    xT = nc.dram_tensor("xT", [D, T], bf16, kind="ExternalInput").ap()
    wq = nc.dram_tensor("wq", [D, G], bf16, kind="ExternalInput").ap()
    wk = nc.dram_tensor("wk", [D, G], bf16, kind="ExternalInput").ap()
    wv = nc.dram_tensor("wv", [D, G], bf16, kind="ExternalInput").ap()
    bqkv = nc.dram_tensor("bqkv", [3, G], f32, kind="ExternalInput").ap()
    wp = nc.dram_tensor("wp", [G, D], bf16, kind="ExternalInput").ap()
    ones = nc.dram_tensor("ones", [128, NH], f32, kind="ExternalInput").ap()
    outT = nc.dram_tensor("outT", [D, T], bf16, kind="ExternalOutput").ap()

    W_APS = {0: wq, 1: wk, 2: wv}
    # xT viewed as [128, CC, T] for one-DMA resident load
    xT_v = xT.rearrange("(c p) t -> p c t", p=128)
    wp_v = wp.rearrange("(c p) d -> p c d", p=128)

    with tile.TileContext(nc) as tc:
        with (
            tc.tile_pool(name="store", bufs=GC) as store,
            tc.tile_pool(name="vaugp", bufs=TB) as vaugp,
            tc.tile_pool(name="xres", bufs=1) as xres,
            tc.tile_pool(name="misc", bufs=4) as misc,
            tc.tile_pool(name="nrm", bufs=3) as nrm,
            tc.tile_pool(name="stage", bufs=4) as stage,
            tc.tile_pool(name="pm", bufs=2, space="PSUM") as pm,
            tc.tile_pool(name="pq", bufs=2, space="PSUM") as pq,
            tc.tile_pool(name="pvp", bufs=2, space="PSUM") as pvp,
        ):
            ctx_t = [
                store.tile([128, T], bf16, tag="ctx", name=f"ctx{i}")
                for i in range(GC)
            ]
            vaug_tb = [
                vaugp.tile(
                    [128, 4, NH, 65], bf16, tag="vaug", name=f"vaug{i}"
                )
                for i in range(TB)
            ]

            def vaug_sl(j):
                return vaug_tb[j // 4][:, j % 4, :, :]
            ones_bf = misc.tile([128, NH], bf16, tag="ones16")
            nc.gpsimd.dma_start(ones_bf[:], ones)  # f32 -> bf16 cast DMA
            # persistent denominator-gather tiles: engine writes must start
            # at 32-aligned partitions, so hl0 lands on partition 0 and hl1
            # on partition 32; rows 1-31 are memset once to 1.0 so the
            # one-shot [33,512] reciprocal stays finite there (unread)
            den_ab = [
                misc.tile([33, 512], f32, tag=f"den{i}", name=f"den{i}")
                for i in range(3)
            ]
            for t in den_ab:
                nc.vector.memset(t[:], 1.0)
            # warm the ScalarE Exp table during the startup DMA wait
            warm = misc.tile([1, 2], f32, tag="warm")
            nc.vector.memset(warm[:], 0.0)
            nc.scalar.activation(warm[:], warm[:], AF.Exp, scale=0.125)
            # PE warm-up spin: ~70 tiny matmuls on a memset tile keep the
            # PE's HAM activity window busy while the x/weight DMAs land,
            # so real work starts at 2.4GHz instead of 1.2GHz (the clock
            # gate needs ~3.4us of sustained activity to open, and the
            # kernel otherwise idles the PE for its first ~7us)
            wsrc = misc.tile([1, 64], bf16, tag="wsrc")
            nc.vector.memset(wsrc[:], 0.0)
            wps = pq.tile([1, 64], f32, tag="qkv", name="wps")
            for _ in range(70):
                nc.tensor.matmul(wps[:], wsrc[:, 0:1], wsrc[:], start=True, stop=True)

            # resident x^T, one tile per 512-token block so chains gate
            # on per-block DMA completion; block 0 is split into two halves
            # so the very first v-chain starts after a 4KB/partition DMA
            x0h = [
                xres.tile([128, CC, 256], bf16, tag=f"x0h{i}", name=f"x0h{i}")
                for i in range(2)
            ]
            x_tb = [None] + [
                xres.tile([128, CC, 512], bf16, tag=f"x{h}", name=f"x{h}")
                for h in range(1, TB)
            ]

            def x_sl(tb, i):
                """x slice for key-chunk i (128 tokens) of block tb."""
                if tb == 0:
                    return x0h[i // 2][:, :, (i % 2) * 128 : (i % 2) * 128 + 128]
                return x_tb[tb][:, :, i * 128 : (i + 1) * 128]

            for j in range(TCH):
                nc.vector.tensor_copy(vaug_sl(j)[:, :, 64], ones_bf[:])

            kt_cur, qt_cur = [None], [None]
            kt_nxt, qt_nxt = [None], [None]

            attn_pools = (
                tc.tile_pool(name="pw", bufs=7),
                tc.tile_pool(name="pwproj", bufs=1),
                tc.tile_pool(name="pkq", bufs=2),
                tc.tile_pool(name="pp", bufs=20),
                tc.tile_pool(name="pbias", bufs=16),
            )
            pwp, pwpr, pkq, ppool, pbias = (
                pl.__enter__() for pl in attn_pools
            )

            def load_w(o, p):
                """One DMA for all CC chunks of weight o, pair p: tile
                [128, CC, 128]; slice [:, cc, :] is the lhsT for chunk cc."""
                w_ap = W_APS[o].rearrange("(c p) g -> p c g", p=128)
                t = pwp.tile([128, CC, 128], bf16, tag="w", name=f"w{o}_{p}")
                nc.sync.dma_start(
                    t[:], w_ap[:, :, p * 128 : (p + 1) * 128]
                )
                bt = pbias.tile([128, 1], f32, tag="bias", name=f"b{o}_{p}")
                nc.sync.dma_start(bt[:], bqkv[o, p * 128 : (p + 1) * 128])
                return t, bt

            def make_fillers(p):
                """Projection work for pair p: per tb, a q-chain, k-chain
                (token-moving) and a v-group (4 key-chunks, x-stationary,
                natural layout)."""
                wv_t, _ = load_w(2, p)
                wk_t, bt_k = load_w(1, p)
                wq_t, bt_q = load_w(0, p)
                kt = pkq.tile([128, T], bf16, tag="kt", name=f"kt{p}")
                qt = pkq.tile([128, T], bf16, tag="qt", name=f"qt{p}")
                kt_nxt[0], qt_nxt[0] = kt, qt
                h0 = 2 * p

                def qk_chain(o, tb):
                    def f():
                        wt = (wk_t, wq_t)[o]
                        ps = pq.tile(
                            [128, 512], f32, tag="qkv", name=f"ps{p}_{o}_{tb}"
                        )
                        if tb == 0:
                            for half in range(2):
                                for cc in range(CC):
                                    nc.tensor.matmul(
                                        ps[:, half * 256 : (half + 1) * 256],
                                        wt[:, cc, :],
                                        x0h[half][:, cc, :],
                                        start=(cc == 0),
                                        stop=(cc == CC - 1),
                                    )
                        else:
                            for cc in range(CC):
                                nc.tensor.matmul(
                                    ps[:],
                                    wt[:, cc, :],
                                    x_tb[tb][:, cc, :],
                                    start=(cc == 0),
                                    stop=(cc == CC - 1),
                                )
                        dst = (kt, qt)[o]
                        bt = (bt_k, bt_q)[o]
                        nc.vector.tensor_scalar_add(
                            dst[:, tb * 512 : (tb + 1) * 512], ps[:], bt[:]
                        )

                    return f

                def v_group(tb):
                    def f():
                        vps = pq.tile(
                            [128, 4, 128], f32, tag="qkv", name=f"vps{p}_{tb}"
                        )
                        for i in range(4):
                            xsl = x_sl(tb, i)
                            for cc in range(CC):
                                nc.tensor.matmul(
                                    vps[:, i, :],
                                    xsl[:, cc, :],
                                    wv_t[:, cc, :],
                                    start=(cc == 0),
                                    stop=(cc == CC - 1),
                                )
                        dst = vaug_tb[tb][:, :, h0 : h0 + 2, 0:64]
                        src = vps[:].rearrange("p k (h c) -> p k h c", c=64)
                        nc.vector.tensor_copy(dst, src)

                    return f

                out = []
                for tb in range(TB):
                    out.append(v_group(tb))
                    out.append(qk_chain(0, tb))
                    out.append(qk_chain(1, tb))
                return out

            fillers = deque()
            pv_backlog = deque()  # PV matmul closures (32 per block)
            epi_backlog = deque()  # released epilogue (+pair-3 proj) closures
            epi_queue = deque()  # per-block closure lists awaiting release

            def s_loop(
                p, qb, mm_rate=2, epi_lo=0, epi_rate=1, epi_stride=1,
                inline_pv=None,
            ):
                """S + exp for (pair p, q-block qb), weaving in backlogged
                PV matmuls (mm_rate per j), the delayed normalization
                epilogue (+pair-3 proj chains; epi_rate per j from j=epi_lo),
                and one projection filler at j%4==1. exp runs on ScalarE
                except DVE_JS chunks (VectorE Schraudolph fast-exp).
                inline_pv: callback (j, P_tiles) for the final q-block."""
                kt, qt = kt_cur[0], qt_cur[0]
                qs = slice(qb * 512, (qb + 1) * 512)
                # final block: split exp evenly so ACT+DVE drain together
                dve_js = DVE_JS if inline_pv is None else tuple(range(1, TCH, 2))
                P_tiles = []
                for j in range(TCH):
                    sp = pm.tile(
                        [128, 1024], f32, tag="mm", name=f"sp{p}_{qb}_{j}"
                    )
                    for hl in range(2):
                        rows = slice(hl * 64, hl * 64 + 64)
                        nc.tensor.matmul(
                            sp[:, hl * 512 : (hl + 1) * 512],
                            kt[rows, j * 128 : (j + 1) * 128],
                            qt[rows, qs],
                            start=True,
                            stop=True,
                            tile_position=(hl * 64, 0),
                        )
                    P = ppool.tile(
                        [128, 1024], bf16, tag="p", name=f"P{p}_{qb}_{j}"
                    )
                    if j in dve_js:
                        nc.vector.tensor_scalar(
                            P[:].bitcast(i16),
                            sp[:],
                            SCHR_SLOPE,
                            SCHR_C,
                            Alu.mult,
                            Alu.add,
                        )
                    else:
                        nc.scalar.activation(P[:], sp[:], AF.Exp, scale=0.125)
                    P_tiles.append(P)
                    if inline_pv is not None:
                        inline_pv(j, P_tiles)
                    for _ in range(mm_rate):
                        if pv_backlog:
                            pv_backlog.popleft()()
                    if j >= epi_lo and (j - epi_lo) % epi_stride == 0:
                        for _ in range(epi_rate):
                            if epi_backlog:
                                epi_backlog.popleft()()
                    if j % 4 == 1 and fillers:
                        fillers.popleft()()
                return P_tiles

            def make_pv(p, qb, P_tiles):
                """P@V accumulation chains for (p, qb) plus the deferred
                normalization epilogue. Returns (mm_items, epi_items);
                epi items: ScalarE den copies -> DVE reciprocal [2,512] ->
                f32 PE broadcast matmul -> ScalarE PSUM->SBUF copy -> DVE
                norm multiplies reading the PV PSUM directly."""
                qs = slice(qb * 512, (qb + 1) * 512)
                pvt = [
                    pvp.tile([65, 512], f32, tag="pv", name=f"pv{p}_{qb}_{i}")
                    for i in range(2)
                ]

                def mk_mm(hl, j):
                    def f():
                        hg = 2 * p + hl
                        nc.tensor.matmul(
                            pvt[hl][0:65, :],
                            vaug_sl(j)[:, hg, :],
                            P_tiles[j][:, hl * 512 : (hl + 1) * 512],
                            start=(j == 0),
                            stop=(j == TCH - 1),
                        )

                    return f

                box = {}
                den = den_ab[(p * TB + qb) % 3]

                def e_drain():
                    # ScalarE PSUM->SBUF drains free the PV accumulator
                    # banks immediately -- the next-next block's PV start
                    # no longer WAR-waits on the reciprocal->broadcast->
                    # norm chain
                    pvs = [
                        nrm.tile([65, 512], f32, tag=f"pvs{i}", name=f"pvs{i}")
                        for i in range(2)
                    ]
                    for hl in range(2):
                        nc.scalar.copy(pvs[hl][:], pvt[hl][:])
                    box["pvs"] = pvs

                def e_den():
                    pvs = box["pvs"]
                    nc.vector.tensor_copy(den[0:1, :], pvs[0][64:65, :])
                    nc.vector.tensor_copy(den[32:33, :], pvs[1][64:65, :])

                def e_recip():
                    rcp = nrm.tile([33, 512], f32, tag="rcp")
                    nc.vector.reciprocal(rcp[:], den[:])
                    # gpsimd's broadcast ucode reads the source on Q7 core
                    # 0 (partitions 0-15), so hl1's reciprocal row hops to
                    # partition 0 via ScalarE
                    rcp_b = nrm.tile([1, 512], f32, tag="rcpb")
                    nc.scalar.copy(rcp_b[:], rcp[32:33, :])
                    box["rcp"], box["rcp_b"] = rcp, rcp_b

                def e_bcast():
                    # gpsimd runs ONLY this op -> its ucode library loads
                    # once, no LOAD/UNLOAD swapping
                    rsb0 = nrm.tile([64, 512], f32, tag="rsb0")
                    rsb1 = nrm.tile([64, 512], f32, tag="rsb1")
                    nc.gpsimd.partition_broadcast(rsb0[:], box["rcp"][0:1, :])
                    nc.gpsimd.partition_broadcast(rsb1[:], box["rcp_b"][:])
                    box["rsb"] = (rsb0, rsb1)

                def e_norm():
                    # stays on DVE: a gpsimd variant forced ucode lib
                    # swaps against partition_broadcast and serialized
                    # the whole epilogue (+200us kernel-wide)
                    for hl in range(2):
                        nc.vector.tensor_tensor(
                            ctx_t[p][hl * 64 : hl * 64 + 64, qs],
                            box["pvs"][hl][0:64, :],
                            box["rsb"][hl][:],
                            Alu.mult,
                        )

                mm_items = []
                for j in range(TCH):
                    for hl in range(2):
                        mm_items.append(mk_mm(hl, j))
                epi_items = [e_drain, e_den, e_recip, e_bcast, e_norm]
                return mm_items, epi_items

            wpt_box = [None]

            def load_wproj():
                wpt_box[0] = pwpr.tile(
                    [128, GC, D], bf16, tag="wproj", name="wpt"
                )
                nc.sync.dma_start(wpt_box[0][:], wp_v)

            def proj_chain(oc, tb):
                def f():
                    wpt = wpt_box[0]
                    ps = pq.tile(
                        [128, 512], f32, tag="qkv", name=f"cps{oc}_{tb}"
                    )
                    for cc in range(GC):
                        nc.tensor.matmul(
                            ps[:],
                            wpt[:, cc, oc * 128 : (oc + 1) * 128],
                            ctx_t[cc][:, tb * 512 : (tb + 1) * 512],
                            start=(cc == 0),
                            stop=(cc == GC - 1),
                        )
                    ost = stage.tile([128, 512], bf16, tag="ost")
                    if tb == TB - 1:
                        nc.scalar.copy(ost[:], ps[:])  # tail: ScalarE is idle
                    else:
                        nc.vector.tensor_copy(ost[:], ps[:])
                    nc.sync.dma_start(
                        outT[
                            oc * 128 : (oc + 1) * 128, tb * 512 : (tb + 1) * 512
                        ],
                        ost[:],
                    )

                return f

            # ---------------- pipeline ----------------
            fillers0 = make_fillers(0)  # issues pair-0 weight DMAs first
            # weights/biases ride the sync queue; x spreads over the
            # scalar/gpsimd queues plus sync (behind the small weight
            # DMAs) so all four blocks land by ~5us
            nc.scalar.dma_start(x0h[0][:], xT_v[:, :, 0:256])
            nc.gpsimd.dma_start(x0h[1][:], xT_v[:, :, 256:512])
            x_queues = (None, nc.scalar, nc.gpsimd, nc.sync)
            for h in range(1, TB):
                sl = slice(h * 512, (h + 1) * 512)
                x_queues[h].dma_start(x_tb[h][:], xT_v[:, :, sl])
            for f in fillers0:
                f()
            kt_cur[0], qt_cur[0] = kt_nxt[0], qt_nxt[0]
            for p in range(GC):
                if p + 1 < GC:
                    fillers.extend(make_fillers(p + 1))
                if p == 2:
                    load_wproj()
                for qb in range(TB):
                    last = p == GC - 1 and qb == TB - 1
                    # release delayed closure lists: pairs 0-2 keep them
                    # two blocks back (popped 1/j from j=0; their PV mms
                    # finished issuing a block earlier). Pair 3 drains PV
                    # at 4/j (done by j=7) and pops epi+proj 2/j from j=8,
                    # one block back, so proj(3,tb) follows norm(3,tb)
                    # inside the same deque.
                    if p < GC - 1:
                        if len(epi_queue) >= 2:
                            epi_backlog.extend(epi_queue.popleft())
                        rates = dict(mm_rate=2, epi_lo=0, epi_rate=1, epi_stride=3)
                    else:
                        while epi_queue:
                            epi_backlog.extend(epi_queue.popleft())
                        rates = dict(mm_rate=4, epi_lo=8, epi_rate=2, epi_stride=1)
                    if not last:
                        P_tiles = s_loop(p, qb, **rates)
                        mm_items, epi_items = make_pv(p, qb, P_tiles)
                        pv_backlog.extend(mm_items)
                        if p == GC - 1:
                            # ctx(3,qb) projections ride the same deque,
                            # strictly after norm(3,qb)
                            epi_items = epi_items + [
                                proj_chain(oc, qb) for oc in range(CC)
                            ]
                        epi_queue.append(epi_items)
                        continue
                    # final q-block: PV(3,2) drains at 4/j (done j=7),
                    # epi(3,2)+proj(tb2) pop 2/j from j=8; own PV inlines
                    # from j=5 (after norm(3,1) issued in the previous
                    # block, freeing its pvp slot); epilogue at j=15,
                    # proj(tb3) in the tail
                    pv_state = {}

                    def inline(j, P_tiles):
                        if j == 5:
                            mm_items, epi_items = make_pv(p, qb, P_tiles)
                            for jj in range(TCH):
                                pv_state[jj] = mm_items[2 * jj : 2 * jj + 2]
                            pv_state["epi"] = epi_items
                            for jj in range(5):
                                for f in pv_state[jj]:
                                    f()
                        if j >= 5:
                            for f in pv_state[j]:
                                f()
                        if j == TCH - 1:
                            for f in pv_state["epi"]:
                                f()

                    s_loop(p, qb, inline_pv=inline, **rates)
                if p + 1 < GC:
                    kt_cur[0], qt_cur[0] = kt_nxt[0], qt_nxt[0]
            # ---------------- tail: remaining proj chains ----------------
            while epi_backlog:
                epi_backlog.popleft()()
            for oc in range(CC):
                proj_chain(oc, TB - 1)()
            for pl in reversed(attn_pools):
                pl.__exit__(None, None, None)

    nc.compile()
    return nc


def _get_nc():
    if "nc" not in _cache:
        _cache["nc"] = _build()
    return _cache["nc"]


def make_in_maps(x, w_qkv, b_qkv, w_proj):
    """Host-side sharding: per-core input dict."""
    BF = ml_dtypes.bfloat16
    x = np.asarray(x, dtype=np.float32)
    w_qkv = np.asarray(w_qkv, dtype=np.float32)
    b_qkv = np.asarray(b_qkv, dtype=np.float32)
    ones = np.ones((128, NH), dtype=np.float32)
    in_maps = []
    for c in range(NCORES):
        b, g = divmod(c, 2)
        sl = slice(g * G, (g + 1) * G)
        in_maps.append(
            {
                "xT": np.ascontiguousarray(x[b].T).astype(BF),
                "wq": np.ascontiguousarray(w_qkv[:, 0 * D : 1 * D][:, sl]).astype(BF),
                "wk": np.ascontiguousarray(w_qkv[:, 1 * D : 2 * D][:, sl]).astype(BF),
                "wv": np.ascontiguousarray(w_qkv[:, 2 * D : 3 * D][:, sl]).astype(BF),
                "bqkv": np.stack(
                    [
                        b_qkv[0 * D : 1 * D][sl],
                        b_qkv[1 * D : 2 * D][sl],
                        b_qkv[2 * D : 3 * D][sl],
                    ]
                ).astype(np.float32),
                "wp": np.ascontiguousarray(
                    np.asarray(w_proj, np.float32)[sl, :]
                ).astype(BF),
                "ones": ones,
            }
        )
    return in_maps


def unshard(results, b_proj):
    b_proj = np.asarray(b_proj, dtype=np.float32)
    out = np.empty((B, T, D), dtype=np.float32)
    for b in range(B):
        s = (
            results[2 * b]["outT"].astype(np.float32)
            + results[2 * b + 1]["outT"].astype(np.float32)
        )  # [D, T]
        out[b] = s.T + b_proj
    return out


def kernel(x, w_qkv, b_qkv, w_proj, b_proj):
    from concourse.bass_utils import run_bass_kernel_spmd

    nc = _get_nc()
    in_maps = make_in_maps(x, w_qkv, b_qkv, w_proj)
    res = run_bass_kernel_spmd(nc, in_maps, core_ids=list(range(NCORES)))
    return unshard(res.results, b_proj)


# revision 35
# speedup vs baseline: 1.5212x; 1.0530x over previous
"""Trainium2 Bass kernel v8 for nn_Attention_72224170050112.

Multi-head attention (B=4, T=2048, D=1024, H=16, HD=64) on 8 NeuronCores.
Sharding: 4-way data-parallel over batch x 2-way tensor-parallel over heads
(core c: batch c//2, head-group c%2). Host sums the two w_proj partials per
batch and adds b_proj.

v8 changes vs v2.1 (479-574us; trace: PE busy only 65%, HAM re-throttled
to 1.2GHz at every block boundary, 85us serial gpsimd norm chain in the
tail). Measured 437us, rel err 1.22e-2.
- Normalization rework: per block, ScalarE drains the two PV [65,512]
  PSUM accumulators to SBUF (freeing the banks immediately, so the next
  PV start never WAR-waits on the rest of the chain), DVE gathers the
  two 'ones'-row denominators into rows 0/32 of a persistent [33,512]
  tile (engine writes need 32-aligned partition starts; rows 1-31 are
  memset once so the reciprocal stays finite there), ONE DVE reciprocal
  covers both head-halves, two gpsimd partition_broadcasts expand the
  reciprocal rows (gpsimd runs only this op -> its ucode lib loads once,
  no LOAD/UNLOAD thrash; its Q7 core 0 reads the source, so hl1's row
  first hops to partition 0 via ScalarE), and two DVE multiplies write
  ctx. Failed variants kept for the record: broadcasting via a tiny f32
  PE matmul (fp32 LOW_HIGH pairs cost 34us of PE and stalled the whole
  in-order PE stream ~4us/block waiting on the reciprocal -- the PE
  reorders only LDWEIGHTS); norms on gpsimd (ucode lib swaps against
  partition_broadcast serialized the epilogue, +200us).
- Re-paced weaving: PV matmuls pop 2 per j-chunk across the whole next
  s_loop (PE never runs dry at block boundaries -> HAM stays at 2.4GHz);
  the 5-item epilogue pops every 3rd j-chunk two blocks later, so the
  3.3us reciprocal sits mid-block in DVE's in-order queue instead of
  blocking the next pair's qt/kt bias-adds at the boundary; fillers pop
  at j%4==1. Pair 3 drains PV at 4/j (done by j=7) and pops epilogue +
  projection chains 2/j from j=8, one block back, so proj(3,tb) follows
  norm(3,tb) inside the same deque.
- Projection chains spread across the last pair: proj(tb) weaves into
  s_loop(3,tb+1) right after ctx(3,tb) is normalized, instead of 24+8
  chains serialized after the final block. wproj DMA moves to pair 2.
- exp split 11/16 ScalarE (true Exp) + 5/16 DVE (Schraudolph bf16
  bit-trick) at j in (2,5,8,11,14); final block splits 8/8.
- A ~70-matmul PE warm-up spin on a memset tile keeps the HAM activity
  window busy while the startup x/weight DMAs land; x rides the scalar/
  gpsimd/sync queues so all four 512-token blocks arrive by ~5us.
"""

import numpy as np
import ml_dtypes

B, T, D, H = 4, 2048, 1024, 16
HD = D // H  # 64
NCORES = 8
G = D // 2  # feature dims per head-group = 512
NH = H // 2  # heads per core = 8
CC = D // 128  # 8 contraction chunks for QKV
GC = G // 128  # 4 head pairs per core
TB = T // 512  # 4 t/q blocks
TCH = T // 128  # 16 t-chunks (attention k chunks)

LOG2E = 1.4426950408889634
SCHR_SLOPE = 128.0 * LOG2E / 8.0
SCHR_C = 128.0 * 127.0 - 7.0 + 0.5  # min-max wiggle correction, trunc->round
DVE_JS = (2, 5, 8, 11, 14)  # j-chunks whose exp runs on VectorE (5/16)

_cache = {}


def _build():
    from collections import deque

    import concourse.bacc as bacc
    import concourse.tile as tile
    from concourse import mybir

    dt = mybir.dt
    f32, bf16, i16 = dt.float32, dt.bfloat16, dt.int16
    AF = mybir.ActivationFunctionType
    Alu = mybir.AluOpType

    nc = bacc.Bacc(
        "TRN2",
        target_bir_lowering=False,
        debug=False,
        enable_asserts=True,
        num_devices=NCORES,
    )
    xT = nc.dram_tensor("xT", [D, T], bf16, kind="ExternalInput").ap()
    wq = nc.dram_tensor("wq", [D, G], bf16, kind="ExternalInput").ap()
    wk = nc.dram_tensor("wk", [D, G], bf16, kind="ExternalInput").ap()
    wv = nc.dram_tensor("wv", [D, G], bf16, kind="ExternalInput").ap()
    bqkv = nc.dram_tensor("bqkv", [3, G], f32, kind="ExternalInput").ap()
    wp = nc.dram_tensor("wp", [G, D], bf16, kind="ExternalInput").ap()
    ones = nc.dram_tensor("ones", [128, NH], f32, kind="ExternalInput").ap()
    outT = nc.dram_tensor("outT", [D, T], bf16, kind="ExternalOutput").ap()

    W_APS = {0: wq, 1: wk, 2: wv}
    # xT viewed as [128, CC, T] for one-DMA resident load
    xT_v = xT.rearrange("(c p) t -> p c t", p=128)
    wp_v = wp.rearrange("(c p) d -> p c d", p=128)

    with tile.TileContext(nc) as tc:
        with (
            tc.tile_pool(name="store", bufs=GC) as store,
            tc.tile_pool(name="vaugp", bufs=TB) as vaugp,
            tc.tile_pool(name="xres", bufs=1) as xres,
            tc.tile_pool(name="misc", bufs=4) as misc,
            tc.tile_pool(name="nrm", bufs=3) as nrm,
            tc.tile_pool(name="stage", bufs=4) as stage,
            tc.tile_pool(name="pm", bufs=2, space="PSUM") as pm,
            tc.tile_pool(name="pq", bufs=2, space="PSUM") as pq,
            tc.tile_pool(name="pvp", bufs=2, space="PSUM") as pvp,
        ):
            ctx_t = [
                store.tile([128, T], bf16, tag="ctx", name=f"ctx{i}")
                for i in range(GC)
            ]
            vaug_tb = [
                vaugp.tile(
                    [128, 4, NH, 65], bf16, tag="vaug", name=f"vaug{i}"
                )
                for i in range(TB)
            ]

            def vaug_sl(j):
                return vaug_tb[j // 4][:, j % 4, :, :]
            ones_bf = misc.tile([128, NH], bf16, tag="ones16")
            nc.gpsimd.dma_start(ones_bf[:], ones)  # f32 -> bf16 cast DMA
            # persistent denominator-gather tiles: engine writes must start
            # at 32-aligned partitions, so hl0 lands on partition 0 and hl1
            # on partition 32; rows 1-31 are memset once to 1.0 so the
            # one-shot [33,512] reciprocal stays finite there (unread)
            den_ab = [
                misc.tile([33, 512], f32, tag=f"den{i}", name=f"den{i}")
                for i in range(3)
            ]
            for t in den_ab:
                nc.vector.memset(t[:], 1.0)
            # warm the ScalarE Exp table during the startup DMA wait
            warm = misc.tile([1, 2], f32, tag="warm")
            nc.vector.memset(warm[:], 0.0)
            nc.scalar.activation(warm[:], warm[:], AF.Exp, scale=0.125)
            # PE warm-up spin: ~70 tiny matmuls on a memset tile keep the
            # PE's HAM activity window busy while the x/weight DMAs land,
            # so real work starts at 2.4GHz instead of 1.2GHz (the clock
            # gate needs ~3.4us of sustained activity to open, and the
            # kernel otherwise idles the PE for its first ~7us)
            wsrc = misc.tile([1, 64], bf16, tag="wsrc")
            nc.vector.memset(wsrc[:], 0.0)
            wps = pq.tile([1, 64], f32, tag="qkv", name="wps")
            for _ in range(70):
                nc.tensor.matmul(wps[:], wsrc[:, 0:1], wsrc[:], start=True, stop=True)

            # resident x^T, one tile per 512-token block so chains gate
            # on per-block DMA completion; block 0 is split into two halves
            # so the very first v-chain starts after a 4KB/partition DMA
            x0h = [
                xres.tile([128, CC, 256], bf16, tag=f"x0h{i}", name=f"x0h{i}")
                for i in range(2)
            ]
            x_tb = [None] + [
                xres.tile([128, CC, 512], bf16, tag=f"x{h}", name=f"x{h}")
                for h in range(1, TB)
            ]

            def x_sl(tb, i):
                """x slice for key-chunk i (128 tokens) of block tb."""
                if tb == 0:
                    return x0h[i // 2][:, :, (i % 2) * 128 : (i % 2) * 128 + 128]
                return x_tb[tb][:, :, i * 128 : (i + 1) * 128]

            for j in range(TCH):
                nc.vector.tensor_copy(vaug_sl(j)[:, :, 64], ones_bf[:])

            kt_cur, qt_cur = [None], [None]
            kt_nxt, qt_nxt = [None], [None]

            attn_pools = (
                tc.tile_pool(name="pw", bufs=7),
                tc.tile_pool(name="pwproj", bufs=1),
                tc.tile_pool(name="pkq", bufs=2),
                tc.tile_pool(name="pp", bufs=20),
                tc.tile_pool(name="pbias", bufs=16),
            )
            pwp, pwpr, pkq, ppool, pbias = (
                pl.__enter__() for pl in attn_pools
            )

            def load_w(o, p):
                """One DMA for all CC chunks of weight o, pair p: tile
                [128, CC, 128]; slice [:, cc, :] is the lhsT for chunk cc."""
                w_ap = W_APS[o].rearrange("(c p) g -> p c g", p=128)
                t = pwp.tile([128, CC, 128], bf16, tag="w", name=f"w{o}_{p}")
                nc.sync.dma_start(
                    t[:], w_ap[:, :, p * 128 : (p + 1) * 128]
                )
                bt = pbias.tile([128, 1], f32, tag="bias", name=f"b{o}_{p}")
                nc.sync.dma_start(bt[:], bqkv[o, p * 128 : (p + 1) * 128])
                return t, bt

            def make_fillers(p):
                """Projection work for pair p: per tb, a q-chain, k-chain
                (token-moving) and a v-group (4 key-chunks, x-stationary,
                natural layout)."""
                wv_t, _ = load_w(2, p)
                wk_t, bt_k = load_w(1, p)
                wq_t, bt_q = load_w(0, p)
                kt = pkq.tile([128, T], bf16, tag="kt", name=f"kt{p}")
                qt = pkq.tile([128, T], bf16, tag="qt", name=f"qt{p}")
                kt_nxt[0], qt_nxt[0] = kt, qt
                h0 = 2 * p

                def qk_chain(o, tb):
                    def f():
                        wt = (wk_t, wq_t)[o]
                        ps = pq.tile(
                            [128, 512], f32, tag="qkv", name=f"ps{p}_{o}_{tb}"
                        )
                        if tb == 0:
                            for half in range(2):
                                for cc in range(CC):
                                    nc.tensor.matmul(
                                        ps[:, half * 256 : (half + 1) * 256],
                                        wt[:, cc, :],
                                        x0h[half][:, cc, :],
                                        start=(cc == 0),
                                        stop=(cc == CC - 1),
                                    )
                        else:
                            for cc in range(CC):
                                nc.tensor.matmul(
                                    ps[:],
                                    wt[:, cc, :],
                                    x_tb[tb][:, cc, :],
                                    start=(cc == 0),
                                    stop=(cc == CC - 1),
                                )
                        dst = (kt, qt)[o]
                        bt = (bt_k, bt_q)[o]
                        nc.vector.tensor_scalar_add(
                            dst[:, tb * 512 : (tb + 1) * 512], ps[:], bt[:]
                        )

                    return f

                def v_group(tb):
                    def f():
                        vps = pq.tile(
                            [128, 4, 128], f32, tag="qkv", name=f"vps{p}_{tb}"
                        )
                        for i in range(4):
                            xsl = x_sl(tb, i)
                            for cc in range(CC):
                                nc.tensor.matmul(
                                    vps[:, i, :],
                                    xsl[:, cc, :],
                                    wv_t[:, cc, :],
                                    start=(cc == 0),
                                    stop=(cc == CC - 1),
                                )
                        dst = vaug_tb[tb][:, :, h0 : h0 + 2, 0:64]
                        src = vps[:].rearrange("p k (h c) -> p k h c", c=64)
                        nc.vector.tensor_copy(dst, src)

                    return f

                out = []
                for tb in range(TB):
                    out.append(v_group(tb))
                    out.append(qk_chain(0, tb))
                    out.append(qk_chain(1, tb))
                return out

            fillers = deque()
            pv_backlog = deque()  # PV matmul closures (32 per block)
            epi_backlog = deque()  # released epilogue (+pair-3 proj) closures
            epi_queue = deque()  # per-block closure lists awaiting release

            def s_loop(
                p, qb, mm_rate=2, epi_lo=0, epi_rate=1, epi_stride=1,
                inline_pv=None,
            ):
                """S + exp for (pair p, q-block qb), weaving in backlogged
                PV matmuls (mm_rate per j), the delayed normalization
                epilogue (+pair-3 proj chains; epi_rate per j from j=epi_lo),
                and one projection filler at j%4==1. exp runs on ScalarE
                except DVE_JS chunks (VectorE Schraudolph fast-exp).
                inline_pv: callback (j, P_tiles) for the final q-block."""
                kt, qt = kt_cur[0], qt_cur[0]
                qs = slice(qb * 512, (qb + 1) * 512)
                # final block: split exp evenly so ACT+DVE drain together
                dve_js = DVE_JS if inline_pv is None else tuple(range(1, TCH, 2))
                P_tiles = []
                for j in range(TCH):
                    sp = pm.tile(
                        [128, 1024], f32, tag="mm", name=f"sp{p}_{qb}_{j}"
                    )
                    for hl in range(2):
                        rows = slice(hl * 64, hl * 64 + 64)
                        nc.tensor.matmul(
                            sp[:, hl * 512 : (hl + 1) * 512],
                            kt[rows, j * 128 : (j + 1) * 128],
                            qt[rows, qs],
                            start=True,
                            stop=True,
                            tile_position=(hl * 64, 0),
                        )
                    P = ppool.tile(
                        [128, 1024], bf16, tag="p", name=f"P{p}_{qb}_{j}"
                    )
                    if j in dve_js:
                        nc.vector.tensor_scalar(
                            P[:].bitcast(i16),
                            sp[:],
                            SCHR_SLOPE,
                            SCHR_C,
                            Alu.mult,
                            Alu.add,
                        )
                    else:
                        nc.scalar.activation(P[:], sp[:], AF.Exp, scale=0.125)
                    P_tiles.append(P)
                    if inline_pv is not None:
                        inline_pv(j, P_tiles)
                    for _ in range(mm_rate):
                        if pv_backlog:
                            pv_backlog.popleft()()
                    if j >= epi_lo and (j - epi_lo) % epi_stride == 0:
                        for _ in range(epi_rate):
                            if epi_backlog:
                                epi_backlog.popleft()()
                    if j % 4 == 1 and fillers:
                        fillers.popleft()()
                return P_tiles

            def make_pv(p, qb, P_tiles):
                """P@V accumulation chains for (p, qb) plus the deferred
                normalization epilogue. Returns (mm_items, epi_items);
                epi items: ScalarE den copies -> DVE reciprocal [2,512] ->
                f32 PE broadcast matmul -> ScalarE PSUM->SBUF copy -> DVE
                norm multiplies reading the PV PSUM directly."""
                qs = slice(qb * 512, (qb + 1) * 512)
                pvt = [
                    pvp.tile([65, 512], f32, tag="pv", name=f"pv{p}_{qb}_{i}")
                    for i in range(2)
                ]

                def mk_mm(hl, j):
                    def f():
                        hg = 2 * p + hl
                        nc.tensor.matmul(
                            pvt[hl][0:65, :],
                            vaug_sl(j)[:, hg, :],
                            P_tiles[j][:, hl * 512 : (hl + 1) * 512],
                            start=(j == 0),
                            stop=(j == TCH - 1),
                        )

                    return f

                box = {}
                den = den_ab[(p * TB + qb) % 3]

                def e_drain():
                    # ScalarE PSUM->SBUF drains free the PV accumulator
                    # banks immediately -- the next-next block's PV start
                    # no longer WAR-waits on the reciprocal->broadcast->
                    # norm chain
                    pvs = [
                        nrm.tile([65, 512], f32, tag=f"pvs{i}", name=f"pvs{i}")
                        for i in range(2)
                    ]
                    for hl in range(2):
                        nc.scalar.copy(pvs[hl][:], pvt[hl][:])
                    box["pvs"] = pvs

                def e_den():
                    pvs = box["pvs"]
                    nc.vector.tensor_copy(den[0:1, :], pvs[0][64:65, :])
                    nc.vector.tensor_copy(den[32:33, :], pvs[1][64:65, :])

                def e_recip_k(k):
                    # the reciprocal costs ~8 cyc per free-dim element
                    # regardless of partitions; as one [33,512] op it was
                    # a 3.3us head-of-line block in DVE's strict FIFO,
                    # stalling the exps that the next S matmuls' PSUM-slot
                    # WAR waits on. Four [33,128] pieces popped at
                    # different j-slots let exps interleave between them.
                    def f():
                        if k == 0:
                            box["rcp"] = nrm.tile(
                                [33, 512], f32, tag="rcp", name="rcp"
                            )
                        sl = slice(k * 128, (k + 1) * 128)
                        nc.vector.reciprocal(box["rcp"][:, sl], den[:, sl])
                        if k == 3:
                            # gpsimd's broadcast ucode reads the source on
                            # Q7 core 0 (partitions 0-15), so hl1's row
                            # hops to partition 0 via ScalarE
                            rcp_b = nrm.tile([1, 512], f32, tag="rcpb")
                            nc.scalar.copy(rcp_b[:], box["rcp"][32:33, :])
                            box["rcp_b"] = rcp_b

                    return f

                def e_bcast():
                    # gpsimd runs ONLY this op -> its ucode library loads
                    # once, no LOAD/UNLOAD swapping
                    rsb0 = nrm.tile([64, 512], f32, tag="rsb0")
                    rsb1 = nrm.tile([64, 512], f32, tag="rsb1")
                    nc.gpsimd.partition_broadcast(rsb0[:], box["rcp"][0:1, :])
                    nc.gpsimd.partition_broadcast(rsb1[:], box["rcp_b"][:])
                    box["rsb"] = (rsb0, rsb1)

                def e_norm():
                    # stays on DVE: a gpsimd variant forced ucode lib
                    # swaps against partition_broadcast and serialized
                    # the whole epilogue (+200us kernel-wide)
                    for hl in range(2):
                        nc.vector.tensor_tensor(
                            ctx_t[p][hl * 64 : hl * 64 + 64, qs],
                            box["pvs"][hl][0:64, :],
                            box["rsb"][hl][:],
                            Alu.mult,
                        )

                mm_items = []
                for j in range(TCH):
                    for hl in range(2):
                        mm_items.append(mk_mm(hl, j))
                epi_items = [
                    e_drain, e_den,
                    e_recip_k(0), e_recip_k(1), e_recip_k(2), e_recip_k(3),
                    e_bcast, e_norm,
                ]
                return mm_items, epi_items

            wpt_box = [None]

            def load_wproj():
                wpt_box[0] = pwpr.tile(
                    [128, GC, D], bf16, tag="wproj", name="wpt"
                )
                nc.sync.dma_start(wpt_box[0][:], wp_v)

            def proj_chain(oc, tb):
                def f():
                    wpt = wpt_box[0]
                    ps = pq.tile(
                        [128, 512], f32, tag="qkv", name=f"cps{oc}_{tb}"
                    )
                    for cc in range(GC):
                        nc.tensor.matmul(
                            ps[:],
                            wpt[:, cc, oc * 128 : (oc + 1) * 128],
                            ctx_t[cc][:, tb * 512 : (tb + 1) * 512],
                            start=(cc == 0),
                            stop=(cc == GC - 1),
                        )
                    ost = stage.tile([128, 512], bf16, tag="ost")
                    if tb == TB - 1:
                        nc.scalar.copy(ost[:], ps[:])  # tail: ScalarE is idle
                    else:
                        nc.vector.tensor_copy(ost[:], ps[:])
                    nc.sync.dma_start(
                        outT[
                            oc * 128 : (oc + 1) * 128, tb * 512 : (tb + 1) * 512
                        ],
                        ost[:],
                    )

                return f

            # ---------------- pipeline ----------------
            fillers0 = make_fillers(0)  # issues pair-0 weight DMAs first
            # weights/biases ride the sync queue; x spreads over the
            # scalar/gpsimd queues plus sync (behind the small weight
            # DMAs) so all four blocks land by ~5us
            nc.scalar.dma_start(x0h[0][:], xT_v[:, :, 0:256])
            nc.gpsimd.dma_start(x0h[1][:], xT_v[:, :, 256:512])
            x_queues = (None, nc.scalar, nc.gpsimd, nc.sync)
            for h in range(1, TB):
                sl = slice(h * 512, (h + 1) * 512)
                x_queues[h].dma_start(x_tb[h][:], xT_v[:, :, sl])
            for f in fillers0:
                f()
            kt_cur[0], qt_cur[0] = kt_nxt[0], qt_nxt[0]
            for p in range(GC):
                if p + 1 < GC:
                    fillers.extend(make_fillers(p + 1))
                if p == 2:
                    load_wproj()
                for qb in range(TB):
                    last = p == GC - 1 and qb == TB - 1
                    # release delayed closure lists: pairs 0-2 keep them
                    # two blocks back (popped 1/j from j=0; their PV mms
                    # finished issuing a block earlier). Pair 3 drains PV
                    # at 4/j (done by j=7) and pops epi+proj 2/j from j=8,
                    # one block back, so proj(3,tb) follows norm(3,tb)
                    # inside the same deque.
                    if p < GC - 1:
                        if len(epi_queue) >= 2:
                            epi_backlog.extend(epi_queue.popleft())
                        rates = dict(mm_rate=2, epi_lo=0, epi_rate=1, epi_stride=2)
                    else:
                        while epi_queue:
                            epi_backlog.extend(epi_queue.popleft())
                        rates = dict(mm_rate=4, epi_lo=8, epi_rate=2, epi_stride=1)
                    if not last:
                        P_tiles = s_loop(p, qb, **rates)
                        mm_items, epi_items = make_pv(p, qb, P_tiles)
                        pv_backlog.extend(mm_items)
                        if p == GC - 1:
                            # ctx(3,qb) projections ride the same deque,
                            # strictly after norm(3,qb)
                            epi_items = epi_items + [
                                proj_chain(oc, qb) for oc in range(CC)
                            ]
                        epi_queue.append(epi_items)
                        continue
                    # final q-block: PV(3,2) drains at 4/j (done j=7),
                    # epi(3,2)+proj(tb2) pop 2/j from j=8; own PV inlines
                    # from j=5 (after norm(3,1) issued in the previous
                    # block, freeing its pvp slot); epilogue at j=15,
                    # proj(tb3) in the tail
                    pv_state = {}

                    def inline(j, P_tiles):
                        if j == 5:
                            mm_items, epi_items = make_pv(p, qb, P_tiles)
                            for jj in range(TCH):
                                pv_state[jj] = mm_items[2 * jj : 2 * jj + 2]
                            pv_state["epi"] = epi_items
                            for jj in range(5):
                                for f in pv_state[jj]:
                                    f()
                        if j >= 5:
                            for f in pv_state[j]:
                                f()
                        if j == TCH - 1:
                            for f in pv_state["epi"]:
                                f()

                    s_loop(p, qb, inline_pv=inline, **rates)
                if p + 1 < GC:
                    kt_cur[0], qt_cur[0] = kt_nxt[0], qt_nxt[0]
            # ---------------- tail: remaining proj chains ----------------
            while epi_backlog:
                epi_backlog.popleft()()
            for oc in range(CC):
                proj_chain(oc, TB - 1)()
            for pl in reversed(attn_pools):
                pl.__exit__(None, None, None)

    nc.compile()
    return nc


def _get_nc():
    if "nc" not in _cache:
        _cache["nc"] = _build()
    return _cache["nc"]


def make_in_maps(x, w_qkv, b_qkv, w_proj):
    """Host-side sharding: per-core input dict."""
    BF = ml_dtypes.bfloat16
    x = np.asarray(x, dtype=np.float32)
    w_qkv = np.asarray(w_qkv, dtype=np.float32)
    b_qkv = np.asarray(b_qkv, dtype=np.float32)
    ones = np.ones((128, NH), dtype=np.float32)
    in_maps = []
    for c in range(NCORES):
        b, g = divmod(c, 2)
        sl = slice(g * G, (g + 1) * G)
        in_maps.append(
            {
                "xT": np.ascontiguousarray(x[b].T).astype(BF),
                "wq": np.ascontiguousarray(w_qkv[:, 0 * D : 1 * D][:, sl]).astype(BF),
                "wk": np.ascontiguousarray(w_qkv[:, 1 * D : 2 * D][:, sl]).astype(BF),
                "wv": np.ascontiguousarray(w_qkv[:, 2 * D : 3 * D][:, sl]).astype(BF),
                "bqkv": np.stack(
                    [
                        b_qkv[0 * D : 1 * D][sl],
                        b_qkv[1 * D : 2 * D][sl],
                        b_qkv[2 * D : 3 * D][sl],
                    ]
                ).astype(np.float32),
                "wp": np.ascontiguousarray(
                    np.asarray(w_proj, np.float32)[sl, :]
                ).astype(BF),
                "ones": ones,
            }
        )
    return in_maps


def unshard(results, b_proj):
    b_proj = np.asarray(b_proj, dtype=np.float32)
    out = np.empty((B, T, D), dtype=np.float32)
    for b in range(B):
        s = (
            results[2 * b]["outT"].astype(np.float32)
            + results[2 * b + 1]["outT"].astype(np.float32)
        )  # [D, T]
        out[b] = s.T + b_proj
    return out


def kernel(x, w_qkv, b_qkv, w_proj, b_proj):
    from concourse.bass_utils import run_bass_kernel_spmd

    nc = _get_nc()
    in_maps = make_in_maps(x, w_qkv, b_qkv, w_proj)
    res = run_bass_kernel_spmd(nc, in_maps, core_ids=list(range(NCORES)))
    return unshard(res.results, b_proj)
